# revision 1
# baseline (speedup 1.0000x reference)
"""Multi-head attention (B=2, N=2048, D=1024, H=16) sharded over 8 trn2 cores.

Sharding: batch (2) x head-groups (4 groups of 4 heads) = 8 cores.
Each core, for its (batch b, head-group g):
  Q.T/K.T feature-major and V token-major projections of its group,
  S.T = K @ Q.T scores, P.T = exp(S.T / 8),
  ctx'.T = [V | ones].T @ P.T  (ones column yields softmax denominators),
  ctx.T normalized via K=1 broadcast matmul of 1/sums,
  partial O = ctx.T.T @ w_o_g.T  (row-parallel O projection).
Host sums the 4 group partials per batch and adds b_o + b_v @ w_o.T
(b_v contributes exactly b_v to ctx since softmax weights sum to 1, so
it folds into the output bias and the kernel never sees it).

v2 schedule: PE warm-up matmuls at t=0 (HAM ramp), DMA order
K -> Q(first query chunk) -> V -> Q(rest) -> w_o so scores/exp start at
~19us instead of ~38us; V' projection interleaved into chain 0 chasing
xv arrivals; remaining Q/O/norm work drip-fed between attention groups
in ~2-matmul units to cover the ACT exp deficit without delaying the
next group's scores.
"""

import os
import sys

for _p in ("/opt/trn_rl_repo",):
    if _p not in sys.path and os.path.isdir(_p):
        sys.path.insert(0, _p)

import ml_dtypes
import numpy as np

import concourse.bass as bass
import concourse.tile as tile
from concourse import bacc, mybir
from concourse.bass_utils import run_bass_kernel_spmd

F32 = mybir.dt.float32
BF16 = mybir.dt.bfloat16
EXP = mybir.ActivationFunctionType.Exp

B = 2
D = 1024
N_HEADS = 16
DK = 64
N_CORES = 8
N_GROUPS = 4
GF = D // N_GROUPS  # 256 features per group
HPG = N_HEADS // N_GROUPS  # 4 heads per group
PAIRS = HPG // 2  # head pairs (2 heads x 64 feats = 128 partitions)
KC = D // 128  # contraction chunks for the input projections


def build_nc(n_tok: int, loop_k: int = 1):
    import contextlib
    assert n_tok % 512 == 0
    QC = n_tok // 512
    TT = n_tok // 128

    nc = bacc.Bacc("TRN2", target_bir_lowering=False, debug=False,
                   num_devices=N_CORES)

    xqT = nc.dram_tensor("xqT", [D, n_tok], BF16, kind="ExternalInput")
    xkT = nc.dram_tensor("xkT", [D, n_tok], BF16, kind="ExternalInput")
    xvT = nc.dram_tensor("xvT", [D, n_tok], BF16, kind="ExternalInput")
    wqT = nc.dram_tensor("wqT", [D, GF], BF16, kind="ExternalInput")
    wkT = nc.dram_tensor("wkT", [D, GF], BF16, kind="ExternalInput")
    wvT = nc.dram_tensor("wvT", [D, GF], BF16, kind="ExternalInput")
    woT = nc.dram_tensor("woT", [GF, D], BF16, kind="ExternalInput")
    bq2 = nc.dram_tensor("bq2", [128, 2], F32, kind="ExternalInput")
    out_p = nc.dram_tensor("out_p", [n_tok, D], BF16, kind="ExternalOutput")

    def mm(out, lhsT, rhs, **kw):
        nc.tensor.matmul(out, lhsT, rhs, **kw)

    with tile.TileContext(nc) as tc:
      with (tc.For_i(0, loop_k, 1) if loop_k > 1
            else contextlib.nullcontext()):
        with (
            tc.tile_pool(name="weights", bufs=1) as wpool,
            tc.tile_pool(name="acts", bufs=1) as apool,
            tc.tile_pool(name="xs", bufs=1) as xspool,
        ):
            wq_sb = wpool.tile([128, KC * GF], BF16, tag="wq")
            wk_sb = wpool.tile([128, KC * GF], BF16, tag="wk")
            wv_sb = wpool.tile([128, KC * GF], BF16, tag="wv")
            wo_sb = wpool.tile([128, 2 * D], BF16, tag="wo")
            bq_sb = wpool.tile([128, 2], F32, tag="bq")
            ones_sb = wpool.tile([128, 128], BF16, tag="ones")
            nc.vector.memset(ones_sb[:], 1.0)
            # tiny dummy exp: forces the ACT exp-table load at t~0
            warm_sb = wpool.tile([1, 8], F32, tag="warm")
            nc.scalar.activation(warm_sb[:], ones_sb[0:1, 0:8], EXP)

            qt_sb = apool.tile([128, PAIRS * n_tok], BF16, tag="qt")
            kt_sb = apool.tile([128, PAIRS * n_tok], BF16, tag="kt")
            v_sb = apool.tile([128, TT * HPG * 65], BF16, tag="v")
            nc.vector.memset(v_sb[:], 1.0)

            xk_t, xv_t, xq_t = [], [], []
            for pfx, lst in (("xk", xk_t), ("xv", xv_t), ("xq", xq_t)):
                for k in range(KC):
                    t = xspool.tile([128, n_tok], BF16, tag=f"{pfx}{k}",
                                    name=f"{pfx}{k}")
                    lst.append(t)

            # PE warm-up: ~3us of no-dependency matmuls at t=0 so the HAM
            # clock gate is at 8/8 before the first projection matmul.
            with tc.tile_pool(name="pswarm", bufs=1, space="PSUM") as pwu:
                wu = [pwu.tile([128, 128], F32, tag=f"wu{i}", name=f"wu{i}")
                      for i in range(2)]
                for i in range(41):
                    mm(wu[i % 2][:], ones_sb[:], ones_sb[:],
                       start=True, stop=True)

            def load_w(w_dram, w_sb):
                nc.sync.dma_start(
                    w_sb[:].rearrange("p (k f) -> p k f", f=GF),
                    w_dram[:].rearrange("(k p) f -> p k f", p=128))

            # DMA emission order = need order: K stream, Q first chunk,
            # V stream, Q rest, O weights.
            load_w(wkT, wk_sb)
            for k in range(KC):
                nc.sync.dma_start(xk_t[k][:], xkT[k * 128:(k + 1) * 128, :])
            load_w(wqT, wq_sb)
            for k in range(KC):
                nc.sync.dma_start(xq_t[k][:, 0:512],
                                  xqT[k * 128:(k + 1) * 128, 0:512])
            nc.sync.dma_start(bq_sb[:], bq2[:])
            load_w(wvT, wv_sb)
            for k in range(KC):
                nc.sync.dma_start(xv_t[k][:], xvT[k * 128:(k + 1) * 128, :])
            for k in range(KC):
                nc.sync.dma_start(xq_t[k][:, 512:n_tok],
                                  xqT[k * 128:(k + 1) * 128, 512:n_tok])
            nc.sync.dma_start(
                wo_sb[:].rearrange("p (c f) -> p c f", f=D),
                woT[:].rearrange("(c p) f -> p c f", p=128),
            )

            # ---- K projection (both pairs) + Q qc0, k-major, chasing DMA --
            with tc.tile_pool(name="psqk", bufs=5, space="PSUM") as psqk:
                for m in range(2):
                    kps = [psqk.tile([128, 512], F32, tag="pqk",
                                     name=f"kps{m}{qc}") for qc in range(QC)]
                    for k in range(KC):
                        for qc in range(QC):
                            mm(kps[qc][:],
                               wk_sb[:, k * GF + m * 128:k * GF + (m + 1) * 128],
                               xk_t[k][:, qc * 512:(qc + 1) * 512],
                               start=(k == 0), stop=(k == KC - 1))
                    for qc in range(QC):
                        nc.vector.tensor_copy(
                            kt_sb[:, m * n_tok + qc * 512:
                                  m * n_tok + (qc + 1) * 512],
                            kps[qc][:])
                qps = [psqk.tile([128, 512], F32, tag="pqk", name=f"qps{m}")
                       for m in range(2)]
                for k in range(KC):
                    for m in range(2):
                        mm(qps[m][:],
                           wq_sb[:, k * GF + m * 128:k * GF + (m + 1) * 128],
                           xq_t[k][:, 0:512],
                           start=(k == 0), stop=(k == KC - 1))
                for m in range(2):
                    nc.vector.tensor_scalar_add(
                        qt_sb[:, m * n_tok:m * n_tok + 512],
                        qps[m][:], bq_sb[:, m:m + 1])

            # ---- attention chains with drip-fed filler -------------------
            with (
                tc.tile_pool(name="pt",
                             bufs=int(os.environ.get("K2_PT", "16"))) as ptpool,
                tc.tile_pool(name="rcp", bufs=2) as rcpool,
                tc.tile_pool(name="ctx", bufs=1) as ctxpool,
                tc.tile_pool(name="ost", bufs=4) as opool,
                tc.tile_pool(name="psctx", bufs=2, space="PSUM") as psctx,
                tc.tile_pool(name="pss", bufs=2, space="PSUM") as pss,
                tc.tile_pool(name="psbo", bufs=2, space="PSUM") as psbo,
            ):
                ctx_t = {}
                deferred = []   # norm closures from the previous chain

                # --- filler units: a few matmuls of PE work each ---------
                # Q projection slices for qc>=1, split into 2 units of 4
                # accumulation MMs; the last unit evicts with the bias.
                qstate = {}

                def make_q_unit(qc_, m_, ph_):
                    def unit():
                        key = (qc_, m_)
                        if ph_ == 0:
                            qstate[key] = psbo.tile([128, 512], F32,
                                                    tag="bo", name="qp")
                        ps = qstate[key]
                        for k in range(4 * ph_, 4 * ph_ + 4):
                            mm(ps[:],
                               wq_sb[:, k * GF + m_ * 128:
                                     k * GF + (m_ + 1) * 128],
                               xq_t[k][:, qc_ * 512:(qc_ + 1) * 512],
                               start=(k == 0), stop=(k == KC - 1))
                        if ph_ == KC // 4 - 1:
                            nc.vector.tensor_scalar_add(
                                qt_sb[:, m_ * n_tok + qc_ * 512:
                                      m_ * n_tok + (qc_ + 1) * 512],
                                ps[:], bq_sb[:, m_:m_ + 1])
                            del qstate[key]
                    return unit

                # O-projection units: (qc, s, n) = 2 accumulation MMs +
                # eviction; DMA of the token slab after its second half.
                ostate = {}

                def make_o_unit(qc_, s_, n_):
                    def unit():
                        key = (qc_, s_)
                        if n_ == 0:
                            ostate[key] = opool.tile([128, D], BF16,
                                                     tag="ost", name="ost")
                        ost = ostate[key]
                        # final chunk: cps banks are dead after the last
                        # norm, so alternate pools for 4 slots in flight
                        opool_ps = (psctx if qc_ == QC - 1 and s_ % 2
                                    else psbo)
                        op = opool_ps.tile([128, 512], F32,
                                           tag="cps" if opool_ps is psctx
                                           else "bo", name="op")
                        for cp in range(PAIRS):
                            mm(op[:],
                               ctx_t[(cp, qc_)][:, s_ * 128:(s_ + 1) * 128],
                               wo_sb[:, cp * D + n_ * 512:
                                     cp * D + (n_ + 1) * 512],
                               start=(cp == 0), stop=(cp == PAIRS - 1))
                        # tail chunk: ACT is idle after the last exp, so it
                        # takes half the evictions off DVE's back
                        if qc_ == QC - 1 and s_ % 2 == 1:
                            nc.scalar.copy(
                                ost[:, n_ * 512:(n_ + 1) * 512], op[:])
                        else:
                            nc.vector.tensor_copy(
                                ost[:, n_ * 512:(n_ + 1) * 512], op[:])
                        if n_ == 1:
                            nc.sync.dma_start(
                                out_p[qc_ * 512 + s_ * 128:
                                      qc_ * 512 + (s_ + 1) * 128, :],
                                ost[:])
                            del ostate[key]
                    return unit

                q_work = []   # deadline-ordered Q units, grouped by qc
                for qc_ in range(1, QC):
                    for m_ in range(2):
                        for ph_ in range(KC // 4):
                            q_work.append((qc_, 852, make_q_unit(qc_, m_, ph_)))
                o_work = []   # (cost_ns, fn)
                fill_budget = [0.0]

                urgq = [False]

                def emit_filler():
                    urgent_q = urgq[0]
                    # drip-feed the per-slot budget. When the next qc's Q
                    # units are imminent (second pair-chain of the current
                    # qc), bank the budget for them instead of letting
                    # cheaper O units starve them into a boundary burst.
                    while True:
                        if q_work and q_work[0][1] <= fill_budget[0]:
                            _, c, fn = q_work.pop(0)
                        elif urgent_q and q_work:
                            return
                        elif o_work and o_work[0][0] <= fill_budget[0]:
                            c, fn = o_work.pop(0)
                        else:
                            return
                        fill_budget[0] -= c
                        fn()

                def force_q(qc_):
                    while q_work and q_work[0][0] <= qc_:
                        q_work.pop(0)[2]()

                # gpsimd partition_broadcast from a partition-64 source
                # produced garbage on the execution path (reads tile
                # partition 0); the K=1 broadcast matmul is the safe route.
                norm_mm = not os.environ.get("K2_NORM_GPSIMD")

                def make_norm(p_, qc_, cps_):
                    # the very last chain's norm is the tail critical path:
                    # route its copies through ACT (idle after the final
                    # exp) so DVE only carries the reciprocal and multiply
                    tail = (qc_ == QC - 1 and p_ == PAIRS - 1)

                    def norm():
                        for h in range(2):
                            rc = rcpool.tile([128, 512], F32,
                                             tag="rc", name="rc")
                            nc.vector.reciprocal(
                                rc[64:65, :], cps_[h][64:65, :])
                            bb = rcpool.tile([128, 512], F32,
                                             tag="bb", name="bb")
                            if norm_mm:
                                rcb = rcpool.tile([128, 512], BF16,
                                                  tag="rcb", name="rcb")
                                cp = nc.scalar.copy if tail \
                                    else nc.vector.tensor_copy
                                with nc.allow_low_precision(
                                        reason="bf16 recip -> bcast mm"):
                                    cp(rcb[64:65, :], rc[64:65, :])
                                bp = psbo.tile([128, 512], F32, tag="bo",
                                               name="bp")
                                mm(bp[0:64, :], ones_sb[64:65, 0:64],
                                   rcb[64:65, :], start=True, stop=True)
                                cp(bb[0:64, :], bp[0:64, :])
                            else:
                                nc.gpsimd.partition_broadcast(
                                    bb[0:64, :], rc[64:65, :], channels=64)
                            nc.vector.tensor_mul(
                                ctx_t[(p_, qc_)][64 * h:64 * h + 64, :],
                                cps_[h][0:64, :], bb[0:64, :])
                        if p_ == PAIRS - 1:
                            for s_ in range(4):
                                for n_ in range(2):
                                    o_work.append(
                                        (426, make_o_unit(qc_, s_, n_)))
                    return norm

                def v_tile(t):
                    vps = psbo.tile([128, 512], F32, tag="bo", name="vp")
                    vp = vps[:, 0:GF]
                    for k in range(KC):
                        mm(vp,
                           xv_t[k][:, t * 128:(t + 1) * 128],
                           wv_sb[:, k * GF:(k + 1) * GF],
                           start=(k == 0), stop=(k == KC - 1))
                    dst = v_sb[:, t * HPG * 65:(t + 1) * HPG * 65]
                    nc.vector.tensor_copy(
                        dst.rearrange("p (h c) -> p h c", c=65)[:, :, 0:DK],
                        vp.rearrange("p (h c) -> p h c", c=DK))

                # Software pipeline: slot (ci, kt2) emits scores(ci, kt2)
                # then AV of a slot AV_LAG back — exp of a slot runs in the
                # shadow of earlier slots' AV + filler, so AV never waits
                # on ACT and the s2 ring recycles in time.
                AV_LAG = int(os.environ.get("K2_AVLAG", "4"))
                BUDGET = float(os.environ.get("K2_BUDGET", "420"))
                pend = []  # (p, qc, kt2, cps, pt)

                def emit_av(item):
                    p_, qc_, kt2_, cps_, pt_ = item
                    for h in range(2):
                        hh = 2 * p_ + h
                        for half in range(2):
                            kt = 2 * kt2_ + half
                            mm(cps_[h][0:65, :],
                               v_sb[:, kt * HPG * 65 + hh * 65:
                                       kt * HPG * 65 + (hh + 1) * 65],
                               pt_[h][:, half * 512:(half + 1) * 512],
                               start=(kt == 0), stop=(kt == TT - 1))
                        if h == 0:
                            emit_filler()

                for ci in range(QC * PAIRS):
                    qc, p = ci // PAIRS, ci % PAIRS
                    urgq[0] = (p == 1)
                    if p == 0:
                        force_q(qc)
                    ctx_t[(p, qc)] = ctxpool.tile(
                        [128, 512], BF16, tag=f"ctx{p}{qc}",
                        name=f"ctx{p}{qc}")
                    cps = [psctx.tile([128, 512], F32, tag="cps",
                                      name=f"cps{h}") for h in range(2)]
                    for kt2 in range(TT // 2):
                        s2 = [pss.tile([128, 1024], F32, tag="s",
                                       name=f"s2_{h}") for h in range(2)]
                        for h in range(2):
                            for half in range(2):
                                kt = 2 * kt2 + half
                                mm(s2[h][:, half * 512:(half + 1) * 512],
                                   kt_sb[64 * h:64 * h + 64,
                                         p * n_tok + kt * 128:
                                         p * n_tok + (kt + 1) * 128],
                                   qt_sb[64 * h:64 * h + 64,
                                         p * n_tok + qc * 512:
                                         p * n_tok + (qc + 1) * 512],
                                   start=True, stop=True)
                        if ci == 0:
                            v_tile(2 * kt2)
                            v_tile(2 * kt2 + 1)
                        pt = []
                        for h in range(2):
                            ptile = ptpool.tile([128, 1024], BF16, tag="pt")
                            nc.scalar.activation(ptile[:], s2[h][:], EXP,
                                                 scale=1.0 / np.sqrt(DK))
                            pt.append(ptile)
                        if kt2 == AV_LAG and deferred:
                            for cl in deferred:
                                cl()
                            deferred.clear()
                        if ci > 0:
                            fill_budget[0] += BUDGET
                        pend.append((p, qc, kt2, cps, pt))
                        last_chain = ci == QC * PAIRS - 1
                        if last_chain and os.environ.get("K2_TAPER"):
                            # drain the AV pipeline during the final chain's
                            # second half (in place of filler) so little
                            # work remains after the last exp
                            lag = max(1, AV_LAG - max(0, kt2 - 3))
                        else:
                            lag = AV_LAG
                        while len(pend) > lag:
                            emit_av(pend.pop(0))
                        if not (last_chain and os.environ.get("K2_TAPER")):
                            emit_filler()
                    deferred.append(make_norm(p, qc, cps))

                while pend:
                    emit_av(pend.pop(0))
                for cl in deferred:
                    cl()
                deferred.clear()
                fill_budget[0] = 1e9
                emit_filler()

    nc.compile()
    return nc


_NC_CACHE: dict[int, object] = {}


def get_nc(n_tok: int):
    if n_tok not in _NC_CACHE:
        _NC_CACHE[n_tok] = build_nc(n_tok)
    return _NC_CACHE[n_tok]


def make_in_maps(query, key, value, w_q, b_q, w_k, b_k, w_v, b_v, w_o, b_o):
    n_tok = query.shape[1]
    bf16 = ml_dtypes.bfloat16
    xT = {}
    for b in range(B):
        xT[("q", b)] = np.ascontiguousarray(query[b].T.astype(bf16))
        xT[("k", b)] = np.ascontiguousarray(key[b].T.astype(bf16))
        xT[("v", b)] = np.ascontiguousarray(value[b].T.astype(bf16))
    in_maps = []
    for core in range(N_CORES):
        b, g = divmod(core, N_GROUPS)
        gs = slice(g * GF, (g + 1) * GF)
        in_maps.append({
            "xqT": xT[("q", b)],
            "xkT": xT[("k", b)],
            "xvT": xT[("v", b)],
            "wqT": np.ascontiguousarray(w_q[gs, :].T.astype(bf16)),
            "wkT": np.ascontiguousarray(w_k[gs, :].T.astype(bf16)),
            "wvT": np.ascontiguousarray(w_v[gs, :].T.astype(bf16)),
            "woT": np.ascontiguousarray(w_o[:, gs].T.astype(bf16)),
            "bq2": np.ascontiguousarray(
                b_q[gs].reshape(2, 128).T, np.float32),
        })
    return in_maps


def kernel(**inputs):
    query = np.asarray(inputs["query"], np.float32)
    n_tok = query.shape[1]
    nc = get_nc(n_tok)
    b_v = np.asarray(inputs["b_v"], np.float32)
    w_o = np.asarray(inputs["w_o"], np.float32)
    in_maps = make_in_maps(
        query, np.asarray(inputs["key"], np.float32),
        np.asarray(inputs["value"], np.float32),
        np.asarray(inputs["w_q"], np.float32), np.asarray(inputs["b_q"], np.float32),
        np.asarray(inputs["w_k"], np.float32), np.asarray(inputs["b_k"], np.float32),
        np.asarray(inputs["w_v"], np.float32), b_v,
        w_o, np.asarray(inputs["b_o"], np.float32),
    )
    res = run_bass_kernel_spmd(nc, in_maps, core_ids=list(range(N_CORES)))
    out = np.zeros((B, n_tok, D), np.float32)
    for core in range(N_CORES):
        b = core // N_GROUPS
        out[b] += res.results[core]["out_p"].astype(np.float32)
    out += np.asarray(inputs["b_o"], np.float32) + b_v @ w_o.T
    return out



# revision 19
# speedup vs baseline: 1.0889x; 1.0889x over previous
"""Multi-head attention (B=2, N=2048, D=1024, H=16) sharded over 8 trn2 cores.

Sharding: batch (2) x head-groups (4 groups of 4 heads) = 8 cores.
Each core, for its (batch b, head-group g):
  Q.T/K.T feature-major and V token-major projections of its group,
  S.T = K @ Q.T scores (row-tiled: the two heads of a pair run
  concurrently in the 64x128 PE tiling), P.T = exp(S.T / 8),
  ctx'.T = [V | ones].T @ P.T  (ones column yields softmax denominators),
  ctx normalized via reciprocal_approx_fast + gpsimd partition_broadcast,
  partial O = ctx.T.T @ w_o_g.T  (row-parallel O projection).
Host sums the 4 group partials per batch and adds b_o + b_v @ w_o.T.

v3 schedule (ACT-saturation focused):
  - ACT exp is the hard wall (~147us of exp per core); everything else
    is scheduled around keeping it busy.
  - DMA in token-slabs in consumption order; K pair-0 projection and
    Q qc0 chase the DMA so the first exp lands ~9.5us.
  - Chain order (qc,p): (0,0),(1,0),(0,1),(1,1),(2,0),(2,1),(3,0),(3,1)
    so chain 0 only needs pair-0 K/Q; pair-1 K proj + all other Q proj
    drip as filler through later chains.
  - V projection (16 tiles, full width) interleaved 2-per-slot in chain 0.
  - Norm: reciprocal_approx_fast on the PSUM denominator row (~5x faster
    than InstReciprocal), gpsimd partition_broadcast (source must sit on
    partition 0 of its tile), one DVE mul. No PE broadcast matmul, no
    bf16 recip copies.
  - AV software pipeline (AV_LAG slots); K2_TAPER=1 enables last-chain taper.
"""

import os
import sys

for _p in ("/opt/trn_rl_repo",):
    if _p not in sys.path and os.path.isdir(_p):
        sys.path.insert(0, _p)

import ml_dtypes
import numpy as np

import concourse.bass as bass
import concourse.tile as tile
from concourse import bacc, mybir
from concourse.bass_utils import run_bass_kernel_spmd

F32 = mybir.dt.float32
BF16 = mybir.dt.bfloat16
EXP = mybir.ActivationFunctionType.Exp

B = 2
D = 1024
N_HEADS = 16
DK = 64
N_CORES = 8
N_GROUPS = 4
GF = D // N_GROUPS  # 256 features per group
HPG = N_HEADS // N_GROUPS  # 4 heads per group
PAIRS = HPG // 2  # head pairs (2 heads x 64 feats = 128 partitions)
KC = D // 128  # contraction chunks for the input projections


def build_nc(n_tok: int, loop_k: int = 1):
    import contextlib
    assert n_tok % 512 == 0
    QC = n_tok // 512
    TT = n_tok // 128
    NSLAB = n_tok // 512  # token slabs of 512

    nc = bacc.Bacc("TRN2", target_bir_lowering=False, debug=False,
                   num_devices=N_CORES)

    xqT = nc.dram_tensor("xqT", [D, n_tok], BF16, kind="ExternalInput")
    xkT = nc.dram_tensor("xkT", [D, n_tok], BF16, kind="ExternalInput")
    xvT = nc.dram_tensor("xvT", [D, n_tok], BF16, kind="ExternalInput")
    wqT = nc.dram_tensor("wqT", [D, GF], BF16, kind="ExternalInput")
    wkT = nc.dram_tensor("wkT", [D, GF], BF16, kind="ExternalInput")
    wvT = nc.dram_tensor("wvT", [D, GF], BF16, kind="ExternalInput")
    woT = nc.dram_tensor("woT", [GF, D], BF16, kind="ExternalInput")
    bq2 = nc.dram_tensor("bq2", [128, 2], F32, kind="ExternalInput")
    out_p = nc.dram_tensor("out_p", [n_tok, D], BF16, kind="ExternalOutput")

    # chain order: (qc, p)
    CHAINS = [(0, 0), (1, 0), (0, 1), (1, 1), (2, 0), (2, 1), (3, 0), (3, 1)]
    assert QC == 4

    def mm(out, lhsT, rhs, **kw):
        nc.tensor.matmul(out, lhsT, rhs, **kw)

    with tile.TileContext(nc) as tc:
      with (tc.For_i(0, loop_k, 1) if loop_k > 1
            else contextlib.nullcontext()):
        with (
            tc.tile_pool(name="weights", bufs=1) as wpool,
            tc.tile_pool(name="acts", bufs=1) as apool,
            tc.tile_pool(name="xs", bufs=1) as xspool,
        ):
            wq_sb = wpool.tile([128, KC * GF], BF16, tag="wq")
            wk_sb = wpool.tile([128, KC * GF], BF16, tag="wk")
            wv_sb = wpool.tile([128, KC * GF], BF16, tag="wv")
            wo_sb = wpool.tile([128, 2 * D], BF16, tag="wo")
            bq_sb = wpool.tile([128, 2], F32, tag="bq")
            ones_sb = wpool.tile([128, 128], BF16, tag="ones")
            nc.vector.memset(ones_sb[:], 1.0)
            # tiny dummy exp: forces the ACT exp-table load at t~0
            warm_sb = wpool.tile([1, 8], F32, tag="warm")
            nc.scalar.activation(warm_sb[:], ones_sb[0:1, 0:8], EXP)

            qt_sb = apool.tile([128, PAIRS * n_tok], BF16, tag="qt")
            kt_sb = apool.tile([128, PAIRS * n_tok], BF16, tag="kt")
            v_sb = apool.tile([128, TT * HPG * 65], BF16, tag="v")
            nc.vector.memset(v_sb[:], 1.0)

            # combined x tiles: [128, (k chunk, token)] so one DMA moves a
            # whole 1 MB token-slab (issue-bandwidth matters: 8 separate
            # 128 KB DMAs cap at ~200 GB/s of Sync-engine issue rate)
            xk_all = xspool.tile([128, KC * n_tok], BF16, tag="xk")
            xv_all = xspool.tile([128, KC * n_tok], BF16, tag="xv")
            xq_all = xspool.tile([128, KC * n_tok], BF16, tag="xq")

            def xk(k, a, b):
                return xk_all[:, k * n_tok + a:k * n_tok + b]

            def xv(k, a, b):
                return xv_all[:, k * n_tok + a:k * n_tok + b]

            def xq(k, a, b):
                return xq_all[:, k * n_tok + a:k * n_tok + b]

            # PE warm-up: ~3.4us of no-dependency matmuls at t=0 so the
            # HAM clock gate is at 8/8 before the first projection matmul.
            with tc.tile_pool(name="pswarm", bufs=1, space="PSUM") as pwu:
                wu = [pwu.tile([128, 128], F32, tag=f"wu{i}", name=f"wu{i}")
                      for i in range(2)]
                for i in range(int(os.environ.get("K2_WARM", "40"))):
                    mm(wu[i % 2][:], ones_sb[:], ones_sb[:],
                       start=True, stop=True)

            def load_w(w_dram, w_sb):
                nc.sync.dma_start(
                    w_sb[:].rearrange("p (k f) -> p k f", f=GF),
                    w_dram[:].rearrange("(k p) f -> p k f", p=128))

            def dma_slab(dst_all, dram, s):
                nc.sync.dma_start(
                    dst_all[:].rearrange("p (k t) -> p k t", t=n_tok)
                    [:, :, s * 512:(s + 1) * 512],
                    dram[:].rearrange("(k p) t -> p k t", p=128)
                    [:, :, s * 512:(s + 1) * 512])

            # DMA emission order = consumption order.
            load_w(wkT, wk_sb)
            load_w(wqT, wq_sb)
            dma_slab(xk_all, xkT, 0)
            dma_slab(xq_all, xqT, 0)
            nc.sync.dma_start(bq_sb[:], bq2[:])
            dma_slab(xk_all, xkT, 1)
            load_w(wvT, wv_sb)
            dma_slab(xv_all, xvT, 0)
            dma_slab(xk_all, xkT, 2)
            dma_slab(xv_all, xvT, 1)
            dma_slab(xk_all, xkT, 3)
            dma_slab(xq_all, xqT, 1)
            dma_slab(xv_all, xvT, 2)
            dma_slab(xv_all, xvT, 3)
            nc.sync.dma_start(
                wo_sb[:].rearrange("p (c f) -> p c f", f=D),
                woT[:].rearrange("(c p) f -> p c f", p=128),
            )
            dma_slab(xq_all, xqT, 2)
            dma_slab(xq_all, xqT, 3)

            # ---- phase A: K pair-0 slab-0 projection + Q qc0 pair-0 -----
            # (K slabs 1-3 are emitted inside chain-0 slots so the PE FIFO
            # doesn't serialize behind their DMA.)
            with tc.tile_pool(name="psqk", bufs=2, space="PSUM") as psqk:
                kps = psqk.tile([128, 512], F32, tag="pqk", name="kps0")
                for k in range(KC):
                    mm(kps[:],
                       wk_sb[:, k * GF:k * GF + 128],
                       xk(k, 0, 512),
                       start=(k == 0), stop=(k == KC - 1))
                nc.vector.tensor_copy(kt_sb[:, 0:512], kps[:])
                qps = psqk.tile([128, 512], F32, tag="pqk", name="qps0")
                for k in range(KC):
                    mm(qps[:],
                       wq_sb[:, k * GF:k * GF + 128],
                       xq(k, 0, 512),
                       start=(k == 0), stop=(k == KC - 1))
                nc.vector.tensor_scalar_add(
                    qt_sb[:, 0:512], qps[:], bq_sb[:, 0:1])

            # ---- attention chains with drip-fed filler -------------------
            with (
                tc.tile_pool(name="pt",
                             bufs=int(os.environ.get("K2_PT", "16"))) as ptpool,
                tc.tile_pool(name="rcp", bufs=2) as rcpool,
                tc.tile_pool(name="ctx", bufs=1) as ctxpool,
                tc.tile_pool(name="ost", bufs=4) as opool,
                tc.tile_pool(name="psctx", bufs=1, space="PSUM") as psctx,
                tc.tile_pool(name="pss", bufs=2, space="PSUM") as pss,
                tc.tile_pool(name="psbo", bufs=2, space="PSUM") as psbo,
            ):
                ctx_t = {}
                deferred = []   # norm closures from the previous chain

                # --- filler units: one self-contained projection each ----
                # K pair-1 slabs and Q (qc,m) slices, keyed by the first
                # chain index that needs the result. Self-contained (psum
                # allocated and evicted within the unit) so interleaved
                # V/O/K units can't steal the accumulator from the ring.
                def make_proj_unit(kind, m_, s_, half_):
                    # kind: 'q' (xq -> qt_sb cols s_*512+half_*256 of pair
                    #       m_), 'k' (same for kt_sb). Column-split into
                    #       256-wide halves so one unit (~0.9us) fits the
                    #       per-slot PE slack; each is self-contained.
                    w_sb = wq_sb if kind == "q" else wk_sb
                    x_f = xq if kind == "q" else xk

                    def unit():
                        lo = s_ * 512 + half_ * 256
                        ps = psbo.tile([128, 512], F32, tag="bo", name="pp")
                        for k in range(KC):
                            mm(ps[:, 0:256],
                               w_sb[:, k * GF + m_ * 128:
                                    k * GF + (m_ + 1) * 128],
                               x_f(k, lo, lo + 256),
                               start=(k == 0), stop=(k == KC - 1))
                        dst_off = m_ * n_tok + lo
                        if kind == "q":
                            nc.vector.tensor_scalar_add(
                                qt_sb[:, dst_off:dst_off + 256],
                                ps[:, 0:256], bq_sb[:, m_:m_ + 1])
                        else:
                            nc.vector.tensor_copy(
                                kt_sb[:, dst_off:dst_off + 256],
                                ps[:, 0:256])
                    return unit

                # O-projection units: (qc, s, n) = 2 accumulation MMs +
                # eviction; DMA of the token slab after its second half.
                ostate = {}

                def make_o_unit(qc_, s_, n_, tail=False):
                    def unit():
                        key = (qc_, s_)
                        if n_ == 0:
                            ostate[key] = opool.tile([128, D], BF16,
                                                     tag="ost", name="ost")
                        ost = ostate[key]
                        op = psbo.tile([128, 512], F32, tag="bo", name="op")
                        for cp in range(PAIRS):
                            mm(op[:],
                               ctx_t[(cp, qc_)][:, s_ * 128:(s_ + 1) * 128],
                               wo_sb[:, cp * D + n_ * 512:
                                     cp * D + (n_ + 1) * 512],
                               start=(cp == 0), stop=(cp == PAIRS - 1))
                        # tail: ACT is idle after the last exp, so it
                        # takes half the evictions off DVE's back
                        if tail and n_ == 1:
                            nc.scalar.copy(
                                ost[:, n_ * 512:(n_ + 1) * 512], op[:])
                        else:
                            nc.vector.tensor_copy(
                                ost[:, n_ * 512:(n_ + 1) * 512], op[:])
                        if n_ == 1:
                            nc.sync.dma_start(
                                out_p[qc_ * 512 + s_ * 128:
                                      qc_ * 512 + (s_ + 1) * 128, :],
                                ost[:])
                            del ostate[key]
                    return unit

                # proj_work: (need_chain_idx, cost_ns, fn), ordered.
                proj_work = []
                # chain 1 = (qc1, p0) needs qt(qc1, m0)
                # chain 2 = (qc0, p1) needs kt m1 (all slabs) + qt(qc0, m1)
                # chain 3 = (qc1, p1) needs qt(qc1, m1)
                # chain 4/5 = qc2, chain 6/7 = qc3
                # keys are slot indices (ci*8 + kt2) of first use
                needs = [(8, "q", 0, 1), (16, "k", 1, 0), (16, "q", 1, 0),
                         (18, "k", 1, 1), (20, "k", 1, 2), (22, "k", 1, 3),
                         (24, "q", 1, 1), (32, "q", 0, 2), (40, "q", 1, 2),
                         (48, "q", 0, 3), (56, "q", 1, 3)]
                for need_slot, kind, m_, s_ in needs:
                    for half_ in range(2):
                        proj_work.append(
                            (need_slot, 852,
                             make_proj_unit(kind, m_, s_, half_)))
                o_work = []   # (cost_ns, fn)
                fill_budget = [0.0]

                def emit_filler():
                    while True:
                        if proj_work and proj_work[0][1] <= fill_budget[0]:
                            _, c, fn = proj_work.pop(0)
                        elif o_work and o_work[0][0] <= fill_budget[0]:
                            c, fn = o_work.pop(0)
                        else:
                            return
                        fill_budget[0] -= c
                        fn()

                def force_proj(slot_):
                    while proj_work and proj_work[0][0] <= slot_:
                        proj_work.pop(0)[2]()

                def make_norm(p_, qc_, cps_, tail=False):
                    def norm():
                        ctx = ctx_t[(p_, qc_)]
                        # Release the cps banks fast (next chain's AVs WAR
                        # on them): evict the denominator row + the
                        # unnormalized ctx, then normalize in SBUF.
                        # custom-DVE recip mishandles base partition 64,
                        # so the denominator is staged on partition 0.
                        dd = rcpool.tile([1, 1024], F32,
                                         tag="dd", name="dd")
                        nc.vector.tensor_copy(dd[0:1, :], cps_[64:65, :])
                        for h in range(2):
                            nc.vector.tensor_copy(
                                ctx[64 * h:64 * h + 64, :],
                                cps_[0:64, h * 512:(h + 1) * 512])
                        rc = rcpool.tile([1, 1024], F32,
                                         tag="rc", name="rc")
                        nc.vector.reciprocal_approx_fast(
                            rc[0:1, :], dd[0:1, :])
                        for h in range(2):
                            # full-height broadcast so the mul's two SBUF
                            # operands share a base partition (IBIR297)
                            bb = rcpool.tile([128, 512], F32,
                                             tag="bb", name="bb")
                            nc.gpsimd.partition_broadcast(
                                bb[:, :],
                                rc[0:1, h * 512:(h + 1) * 512],
                                channels=128)
                            nc.vector.tensor_mul(
                                ctx[64 * h:64 * h + 64, :],
                                ctx[64 * h:64 * h + 64, :],
                                bb[64 * h:64 * h + 64, :])
                        if (PAIRS - 1, qc_) in ctx_t and \
                                (0, qc_) in ctx_t and p_ == PAIRS - 1:
                            for s_ in range(4):
                                for n_ in range(2):
                                    o_work.append(
                                        (426,
                                         make_o_unit(qc_, s_, n_, tail)))
                    return norm

                def v_tile(t):
                    vps = psbo.tile([128, 512], F32, tag="bo", name="vp")
                    vp = vps[:, 0:GF]
                    for k in range(KC):
                        mm(vp,
                           xv(k, t * 128, (t + 1) * 128),
                           wv_sb[:, k * GF:(k + 1) * GF],
                           start=(k == 0), stop=(k == KC - 1))
                    dst = v_sb[:, t * HPG * 65:(t + 1) * HPG * 65]
                    nc.vector.tensor_copy(
                        dst.rearrange("p (h c) -> p h c", c=65)[:, :, 0:DK],
                        vp.rearrange("p (h c) -> p h c", c=DK))

                v_next = [0]

                def v_emit(upto):
                    while v_next[0] < min(upto, TT):
                        v_tile(v_next[0])
                        v_next[0] += 1

                def k_slab_proj(s):
                    ps = psbo.tile([128, 512], F32, tag="bo", name="ksp")
                    for k in range(KC):
                        mm(ps[:],
                           wk_sb[:, k * GF:k * GF + 128],
                           xk(k, s * 512, (s + 1) * 512),
                           start=(k == 0), stop=(k == KC - 1))
                    nc.vector.tensor_copy(
                        kt_sb[:, s * 512:(s + 1) * 512], ps[:])

                # Software pipeline: slot (ci, kt2) emits scores(ci, kt2)
                # then AV of a slot AV_LAG back.
                AV_LAG = int(os.environ.get("K2_AVLAG", "4"))
                BUDGET = float(os.environ.get("K2_BUDGET", "550"))
                pend = []  # (p, qc, kt2, cps, pt)

                def emit_av(item):
                    p_, qc_, kt2_, cps_, pt_ = item
                    v_emit(2 * kt2_ + 2)  # force V tiles this AV reads
                    for h in range(2):
                        hh = 2 * p_ + h
                        for half in range(2):
                            kt = 2 * kt2_ + half
                            mm(cps_[0:65, h * 512:(h + 1) * 512],
                               v_sb[:, kt * HPG * 65 + hh * 65:
                                       kt * HPG * 65 + (hh + 1) * 65],
                               pt_[h][:, half * 512:(half + 1) * 512],
                               start=(kt == 0), stop=(kt == TT - 1))
                        if h == 0:
                            emit_filler()

                taper = bool(os.environ.get("K2_TAPER"))
                for ci, (qc, p) in enumerate(CHAINS):
                    ctx_t[(p, qc)] = ctxpool.tile(
                        [128, 512], BF16, tag=f"ctx{p}{qc}",
                        name=f"ctx{p}{qc}")
                    cps = psctx.tile([128, 1024], F32, tag="cps",
                                     name="cps")
                    for kt2 in range(TT // 2):
                        force_proj(ci * 8 + kt2)
                        s2 = [pss.tile([128, 1024], F32, tag="s",
                                       name=f"s2_{h}") for h in range(2)]
                        for h in range(2):
                            for half in range(2):
                                kt = 2 * kt2 + half
                                mm(s2[h][:, half * 512:(half + 1) * 512],
                                   kt_sb[64 * h:64 * h + 64,
                                         p * n_tok + kt * 128:
                                         p * n_tok + (kt + 1) * 128],
                                   qt_sb[64 * h:64 * h + 64,
                                         p * n_tok + qc * 512:
                                         p * n_tok + (qc + 1) * 512],
                                   start=True, stop=True)
                        pt = []
                        for h in range(2):
                            ptile = ptpool.tile([128, 1024], BF16, tag="pt")
                            nc.scalar.activation(ptile[:], s2[h][:], EXP,
                                                 scale=1.0 / np.sqrt(DK))
                            pt.append(ptile)
                        if ci == 0:
                            # prefetch the next K slab right when its DMA
                            # lands, then drip V tiles 2 per slot
                            if kt2 in (0, 2, 4):
                                k_slab_proj(kt2 // 2 + 1)
                            if kt2 >= 1:
                                v_emit(2 * kt2)
                        elif ci == 1 and kt2 < 2:
                            # finish remaining V tiles, paced vs xv DMA
                            v_emit(TT - 1 + kt2)
                        if ci > 0:
                            fill_budget[0] += BUDGET
                        pend.append((p, qc, kt2, cps, pt))
                        last_chain = ci == len(CHAINS) - 1
                        if last_chain and taper:
                            # drain the AV pipeline during the final
                            # chain's second half so little work remains
                            # after the last exp
                            lag = max(1, AV_LAG - max(0, kt2 - 3))
                        else:
                            lag = AV_LAG
                        while len(pend) > lag:
                            emit_av(pend.pop(0))
                        if kt2 == AV_LAG - 1 and deferred:
                            # prev chain's last AV just popped: run its
                            # norm now so the cps banks free up a slot
                            # before this chain's first AV pops
                            for cl in deferred:
                                cl()
                            deferred.clear()
                        if not (last_chain and taper):
                            emit_filler()
                    deferred.append(
                        make_norm(p, qc, cps,
                                  tail=(ci == len(CHAINS) - 1)))

                while pend:
                    emit_av(pend.pop(0))
                for cl in deferred:
                    cl()
                deferred.clear()
                fill_budget[0] = 1e9
                emit_filler()

    nc.compile()
    return nc


_NC_CACHE: dict[int, object] = {}


def get_nc(n_tok: int):
    if n_tok not in _NC_CACHE:
        _NC_CACHE[n_tok] = build_nc(n_tok)
    return _NC_CACHE[n_tok]


def make_in_maps(query, key, value, w_q, b_q, w_k, b_k, w_v, b_v, w_o, b_o):
    n_tok = query.shape[1]
    bf16 = ml_dtypes.bfloat16
    xT = {}
    for b in range(B):
        xT[("q", b)] = np.ascontiguousarray(query[b].T.astype(bf16))
        xT[("k", b)] = np.ascontiguousarray(key[b].T.astype(bf16))
        xT[("v", b)] = np.ascontiguousarray(value[b].T.astype(bf16))
    in_maps = []
    for core in range(N_CORES):
        b, g = divmod(core, N_GROUPS)
        gs = slice(g * GF, (g + 1) * GF)
        in_maps.append({
            "xqT": xT[("q", b)],
            "xkT": xT[("k", b)],
            "xvT": xT[("v", b)],
            "wqT": np.ascontiguousarray(w_q[gs, :].T.astype(bf16)),
            "wkT": np.ascontiguousarray(w_k[gs, :].T.astype(bf16)),
            "wvT": np.ascontiguousarray(w_v[gs, :].T.astype(bf16)),
            "woT": np.ascontiguousarray(w_o[:, gs].T.astype(bf16)),
            "bq2": np.ascontiguousarray(
                b_q[gs].reshape(2, 128).T, np.float32),
        })
    return in_maps


def kernel(**inputs):
    query = np.asarray(inputs["query"], np.float32)
    n_tok = query.shape[1]
    nc = get_nc(n_tok)
    b_v = np.asarray(inputs["b_v"], np.float32)
    w_o = np.asarray(inputs["w_o"], np.float32)
    in_maps = make_in_maps(
        query, np.asarray(inputs["key"], np.float32),
        np.asarray(inputs["value"], np.float32),
        np.asarray(inputs["w_q"], np.float32), np.asarray(inputs["b_q"], np.float32),
        np.asarray(inputs["w_k"], np.float32), np.asarray(inputs["b_k"], np.float32),
        np.asarray(inputs["w_v"], np.float32), b_v,
        w_o, np.asarray(inputs["b_o"], np.float32),
    )
    res = run_bass_kernel_spmd(nc, in_maps, core_ids=list(range(N_CORES)))
    out = np.zeros((B, n_tok, D), np.float32)
    for core in range(N_CORES):
        b = core // N_GROUPS
        out[b] += res.results[core]["out_p"].astype(np.float32)
    out += np.asarray(inputs["b_o"], np.float32) + b_v @ w_o.T
    return out
